# revision 3
# baseline (speedup 1.0000x reference)
"""Performer (FAVOR+) attention on 8 Trainium2 NeuronCores via Bass/Tile.

Reference computation (per batch b):
    K = K * mask[:, None];  V = V * mask[:, None]
    kp = exp(K @ w.T - 0.5*||K||^2) / sqrt(M)          # [T, M]
    qp = exp(Q @ w.T - 0.5*||Q||^2) / sqrt(M)          # [T, M]
    D  = qp @ kp.sum(0)                                # [T]
    kptv = V.T @ kp                                    # [DIM, M]
    y  = (qp @ kptv.T) / (D+eps) / (D+eps)             # [T, DIM]

Sharding: 8 cores = 4 batches x 2 halves of T. Each core computes its
(b, T-half) shard end to end; the K-side partial sums (kptv, kp.sum)
are AllReduced between the two cores sharing a batch.

All matmuls run in bf16 (fp32 accumulation in PSUM). exp() and the
final normalization run in fp32.
"""

import math
from contextlib import ExitStack

import numpy as np

B, T, DIM = 4, 4096, 1024
M = DIM // 2          # 512 random features
NCORES = 8
TH = T // 2           # 2048 rows of T per core
EPS = 1e-8
HALF_LN_M = 0.5 * math.log(M)

P = 128               # partitions
NT = TH // P          # 16 t-tiles per core
NTB = TH // 512       # 4 t-blocks of 512
NI = DIM // P         # 8 contraction chunks over dim
NM = M // P           # 4 m-tiles
FREE = 512            # matmul moving/free width (psum bank = 512 f32)

_COMPILED = {}


def _emit(tc, nc, ctx, q, k, v, msk, w, y, replica_groups):
    import concourse.mybir as mybir
    from concourse import masks

    dt = mybir.dt
    f32, bf16 = dt.float32, dt.bfloat16
    AFT = mybir.ActivationFunctionType

    # ---------------- pools ----------------
    res = ctx.enter_context(tc.tile_pool(name="res", bufs=1))
    stream = ctx.enter_context(tc.tile_pool(name="stream", bufs=1))
    dram = ctx.enter_context(tc.tile_pool(name="dram", bufs=1, space="DRAM"))
    # long-lived PSUM: wtx (2 banks) + kd (1 bank) = 3 of 8 banks
    ps_main = ctx.enter_context(tc.tile_pool(name="ps_main", bufs=1, space="PSUM"))

    def rtile(shape, dtype, tag):
        return res.tile(shape, dtype, tag=tag, name=tag)

    def stile(shape, dtype, tag, bufs):
        return stream.tile(shape, dtype, tag=tag, name=tag, bufs=bufs)

    # ---------------- constants ----------------
    identity = rtile([P, P], bf16, "identity")
    masks.make_identity(nc, identity[:])
    ones_col = rtile([P, 1], bf16, "ones_col")
    nc.vector.memset(ones_col[:], 1.0)

    # mask in both layouts
    mask_cols = rtile([P, NT], f32, "mask_cols")   # mask_cols[p, tt] = mask[tt*128+p]
    for tt in range(NT):
        nc.sync.dma_start(
            out=mask_cols[:, tt : tt + 1], in_=msk[tt * P : (tt + 1) * P]
        )
    mask_row = rtile([1, TH], f32, "mask_row")
    nc.sync.dma_start(out=mask_row[:], in_=msk.rearrange("(a t) -> a t", a=1))

    # w^T: [dim, M] in bf16, 8 chunks of [128i, 512m]
    wT = []
    for ic in range(NI):
        wf = stile([P, M], f32, "wf", 2)
        nc.sync.dma_start(
            out=wf[:], in_=w[:, ic * P : (ic + 1) * P].rearrange("m i -> i m")
        )
        wt = rtile([P, M], bf16, f"wT{ic}")
        nc.vector.tensor_copy(wt[:], wf[:])
        wT.append(wt)

    kd_cols = rtile([P, NT], f32, "kd_cols")   # -(0.5*mask*||K_t||^2 + 0.5*ln M)
    qd_cols = rtile([P, NT], f32, "qd_cols")   # -(0.5*||Q_t||^2 + 0.5*ln M)

    # ---------------- x^T load + row-norm helper ----------------
    def load_side(x_ap, cols_out, apply_mask):
        """Strided-load x^T chunks (bf16) and compute the exp bias columns.

        Returns xT[tb][i] bf16 tiles of [128i, 512t]."""
        name = "k" if apply_mask else "q"
        xT = [[None] * NI for _ in range(NTB)]
        for tb in range(NTB):
            xd_psum = ps_main.tile([1, FREE], f32, tag="kd", name="xd_psum", bufs=1)
            for i in range(NI):
                xf = stile([P, FREE], f32, f"{name}f", 3)
                nc.sync.dma_start(
                    out=xf[:],
                    in_=x_ap[
                        tb * FREE : (tb + 1) * FREE, i * P : (i + 1) * P
                    ].rearrange("t i -> i t"),
                )
                xb = stile([P, FREE], bf16, f"{name}T{i}", 2)
                nc.vector.tensor_copy(xb[:], xf[:])
                xT[tb][i] = xb
                xsq = stile([P, FREE], bf16, f"{name}sq", 2)
                nc.scalar.activation(xsq[:], xf[:], AFT.Square)
                nc.tensor.matmul(
                    xd_psum[:],
                    lhsT=ones_col[:],
                    rhs=xsq[:],
                    start=(i == 0),
                    stop=(i == NI - 1),
                )
            # row layout [1, 512]: -0.5 * sum(x^2), optionally masked, - 0.5 ln M
            xdr = stile([1, FREE], f32, f"{name}dr", 2)
            nc.scalar.activation(xdr[:], xd_psum[:], AFT.Copy, scale=-0.5)
            if apply_mask:
                nc.vector.tensor_mul(
                    xdr[:], xdr[:], mask_row[0:1, tb * FREE : (tb + 1) * FREE]
                )
            nc.vector.tensor_scalar_add(xdr[:], xdr[:], -HALF_LN_M)
            # redistribute row -> per-partition columns
            for j in range(4):
                tt = tb * 4 + j
                nc.sync.dma_start(
                    out=cols_out[:, tt : tt + 1],
                    in_=xdr[0:1, j * P : (j + 1) * P],
                )
        return xT

    # ---------------- K side: kp = exp(mask*wtx + bias) ----------------
    kT = load_side(k, kd_cols, apply_mask=True)

    kp = []
    for tt in range(NT):
        tb, j = tt // 4, tt % 4
        pw = ps_main.tile([P, M], f32, tag="wtx", name="pw", bufs=2)
        for i in range(NI):
            nc.tensor.matmul(
                pw[:],
                lhsT=kT[tb][i][:, j * P : (j + 1) * P],
                rhs=wT[i][:],
                start=(i == 0),
                stop=(i == NI - 1),
            )
        kpt = rtile([P, M], bf16, f"kp{tt}")
        nc.scalar.activation(
            kpt[:],
            pw[:],
            AFT.Exp,
            bias=kd_cols[:, tt : tt + 1],
            scale=mask_cols[:, tt : tt + 1],
        )
        kp.append(kpt)

    # ---------------- V: load + mask + cast ----------------
    vb = []
    for tt in range(NT):
        vf = stile([P, DIM], f32, "vf", 2)
        nc.sync.dma_start(out=vf[:], in_=v[tt * P : (tt + 1) * P, :])
        vbt = rtile([P, DIM], bf16, f"vb{tt}")
        nc.vector.tensor_scalar_mul(vbt[:], vf[:], mask_cols[:, tt : tt + 1])
        vb.append(vbt)

    # ---------------- kptv^T[m, n] = sum_t kp[t, m] V[t, n]; ksum[m] ----------------
    cc_in = dram.tile([M, DIM + 1], f32, tag="cc_in", name="cc_in")
    cc_out = dram.tile([M, DIM + 1], f32, tag="cc_out", name="cc_out")

    with tc.tile_pool(name="ps_kptv", bufs=1, space="PSUM") as ps_kptv:
        for mt in range(NM):
            ccs = rtile([P, DIM + 1], f32, f"ccs{mt}")
            for nh in range(DIM // FREE):
                pk = ps_kptv.tile([P, FREE], f32, tag="kptv", name="pk", bufs=2)
                for tt in range(NT):
                    nc.tensor.matmul(
                        pk[:],
                        lhsT=kp[tt][:, mt * P : (mt + 1) * P],
                        rhs=vb[tt][:, nh * FREE : (nh + 1) * FREE],
                        start=(tt == 0),
                        stop=(tt == NT - 1),
                    )
                nc.vector.tensor_copy(ccs[:, nh * FREE : (nh + 1) * FREE], pk[:])
            pks = ps_kptv.tile([P, 1], f32, tag="ksum", name="pks", bufs=1)
            for tt in range(NT):
                nc.tensor.matmul(
                    pks[:],
                    lhsT=kp[tt][:, mt * P : (mt + 1) * P],
                    rhs=ones_col[:],
                    start=(tt == 0),
                    stop=(tt == NT - 1),
                )
            nc.vector.tensor_copy(ccs[:, DIM : DIM + 1], pks[:])
            nc.sync.dma_start(out=cc_in[mt * P : (mt + 1) * P, :], in_=ccs[:])

    # ---------------- AllReduce partial sums between the T-half pair ----------------
    nc.gpsimd.collective_compute(
        "AllReduce",
        mybir.AluOpType.add,
        replica_groups=replica_groups,
        ins=[cc_in.opt()],
        outs=[cc_out.opt()],
    )
    kptvT = []
    ksum = []
    for mt in range(NM):
        ccb = stile([P, DIM + 1], f32, "ccb", 2)
        nc.sync.dma_start(out=ccb[:], in_=cc_out[mt * P : (mt + 1) * P, :])
        kvt = rtile([P, DIM], bf16, f"kptvT{mt}")
        nc.vector.tensor_copy(kvt[:], ccb[:, 0:DIM])
        kptvT.append(kvt)
        ksb = rtile([P, 1], bf16, f"ksum{mt}")
        nc.vector.tensor_copy(ksb[:], ccb[:, DIM : DIM + 1])
        ksum.append(ksb)

    # ---------------- Q side: qp = exp(wtx + bias), then transpose ----------------
    qT = load_side(q, qd_cols, apply_mask=False)

    qpT = [rtile([P, TH], bf16, f"qpT{mt}") for mt in range(NM)]
    with tc.tile_pool(name="ps_tr", bufs=1, space="PSUM") as ps_tr:
        for tt in range(NT):
            tb, j = tt // 4, tt % 4
            pw = ps_main.tile([P, M], f32, tag="wtx", name="pw", bufs=2)
            for i in range(NI):
                nc.tensor.matmul(
                    pw[:],
                    lhsT=qT[tb][i][:, j * P : (j + 1) * P],
                    rhs=wT[i][:],
                    start=(i == 0),
                    stop=(i == NI - 1),
                )
            qpt = stile([P, M], bf16, "qp", 3)
            nc.scalar.activation(qpt[:], pw[:], AFT.Exp, bias=qd_cols[:, tt : tt + 1])
            for mt in range(NM):
                ptr = ps_tr.tile([P, P], bf16, tag="tr", name="ptr", bufs=2)
                nc.tensor.transpose(ptr[:], qpt[:, mt * P : (mt + 1) * P], identity[:])
                nc.vector.tensor_copy(qpT[mt][:, tt * P : (tt + 1) * P], ptr[:])

    # ---------------- y = (qp @ kptv^T) / (D+eps)^2 ----------------
    with tc.tile_pool(name="ps_y", bufs=1, space="PSUM") as ps_y:
        for tt in range(NT):
            py = [
                ps_y.tile([P, FREE], f32, tag="y", name="py", bufs=2)
                for _ in range(DIM // FREE)
            ]
            pD = ps_y.tile([P, 1], f32, tag="D", name="pD", bufs=2)
            for mt in range(NM):
                lhs = qpT[mt][:, tt * P : (tt + 1) * P]
                for nh in range(DIM // FREE):
                    nc.tensor.matmul(
                        py[nh][:],
                        lhsT=lhs,
                        rhs=kptvT[mt][:, nh * FREE : (nh + 1) * FREE],
                        start=(mt == 0),
                        stop=(mt == NM - 1),
                    )
                nc.tensor.matmul(
                    pD[:],
                    lhsT=lhs,
                    rhs=ksum[mt][:],
                    start=(mt == 0),
                    stop=(mt == NM - 1),
                )
            u = stile([P, 1], f32, "u", 2)
            nc.vector.tensor_scalar_add(u[:], pD[:], EPS)
            rv = stile([P, 1], f32, "rv", 2)
            nc.vector.reciprocal(rv[:], u[:])
            r2 = stile([P, 1], f32, "r2", 2)
            nc.vector.tensor_mul(r2[:], rv[:], rv[:])
            ys = stile([P, DIM], f32, "ys", 3)
            for nh in range(DIM // FREE):
                nc.scalar.activation(
                    ys[:, nh * FREE : (nh + 1) * FREE],
                    py[nh][:],
                    AFT.Copy,
                    scale=r2[:],
                )
            nc.sync.dma_start(out=y[tt * P : (tt + 1) * P, :], in_=ys[:])


def build(num_devices=NCORES):
    """Build + compile the per-core SPMD Bass module."""
    import concourse.bacc as bacc
    import concourse.mybir as mybir
    from concourse import tile

    dt = mybir.dt
    f32 = dt.float32

    if num_devices == NCORES:
        replica_groups = [[2 * g, 2 * g + 1] for g in range(NCORES // 2)]
    else:
        replica_groups = [[c] for c in range(num_devices)]

    nc = bacc.Bacc(
        "TRN2",
        target_bir_lowering=False,
        debug=False,
        num_devices=num_devices,
    )
    q = nc.dram_tensor("q", [TH, DIM], f32, kind="ExternalInput").ap()
    k = nc.dram_tensor("k", [TH, DIM], f32, kind="ExternalInput").ap()
    v = nc.dram_tensor("v", [TH, DIM], f32, kind="ExternalInput").ap()
    msk = nc.dram_tensor("msk", [TH], f32, kind="ExternalInput").ap()
    w = nc.dram_tensor("w", [M, DIM], f32, kind="ExternalInput").ap()
    y = nc.dram_tensor("y", [TH, DIM], f32, kind="ExternalOutput").ap()

    with tile.TileContext(nc) as tc:
        with ExitStack() as ctx:
            _emit(tc, nc, ctx, q, k, v, msk, w, y, replica_groups)
    nc.compile()
    return nc


def _get_compiled(num_devices=NCORES):
    if num_devices not in _COMPILED:
        _COMPILED[num_devices] = build(num_devices)
    return _COMPILED[num_devices]


def make_in_maps(Q, K, V, key_padding_mask, w):
    Q = np.asarray(Q, np.float32)
    K = np.asarray(K, np.float32)
    V = np.asarray(V, np.float32)
    mask = np.asarray(key_padding_mask, np.float32)
    w = np.ascontiguousarray(np.asarray(w, np.float32))
    in_maps = []
    for c in range(NCORES):
        b, h = c // 2, c % 2
        sl = slice(h * TH, (h + 1) * TH)
        in_maps.append(
            {
                "q": np.ascontiguousarray(Q[b, sl]),
                "k": np.ascontiguousarray(K[b, sl]),
                "v": np.ascontiguousarray(V[b, sl]),
                "msk": np.ascontiguousarray(mask[b, sl]),
                "w": w,
            }
        )
    return in_maps


def gather_out(results):
    y = np.empty((B, T, DIM), np.float32)
    for c in range(NCORES):
        b, h = c // 2, c % 2
        y[b, h * TH : (h + 1) * TH] = results[c]["y"]
    return y


def kernel(Q, K, V, key_padding_mask, w):
    from concourse.bass_utils import run_bass_kernel_spmd

    nc = _get_compiled()
    in_maps = make_in_maps(Q, K, V, key_padding_mask, w)
    res = run_bass_kernel_spmd(nc, in_maps, list(range(NCORES)))
    return gather_out(res.results)
